# revision 2
# baseline (speedup 1.0000x reference)
# BiLSTM-CRF negative log-likelihood on 8 Trainium2 NeuronCores.
# Self-contained: host prep + Bass/Tile device program + unshard.
#
# Sharding: data-parallel over batch. 64 sequences -> 8 cores x 8 seqs.
# Per core: embedding gather -> 2-layer BiLSTM -> tag projection ->
# CRF partition (log-semiring tree reduction) + gold emission dot.
# Host: gold transition score (pure index math on host-visible inputs),
# final llh assembly and mean.

import numpy as np

VOCAB = 50000
EMB = 256
HID = 256
H2 = 128
NLAYERS = 2
NTAGS = 4
B = 64
S = 512
NCORES = 8
BL = B // NCORES          # sequences per core
TB = 16                   # timesteps per x-projection PSUM block
UB = 32                   # CRF tree: timesteps per lane (q = t // UB)
NEG = -1.0e9

_BUILD_CACHE = {}


# --------------------------------------------------------------------------
# Device program
# --------------------------------------------------------------------------

def build_program(s_len=S, sigma_trick=True, n_devices=NCORES):
    """Builds the per-core Bass program (identical on all cores)."""
    import concourse.bacc as bacc
    import concourse.bass as bass
    import concourse.tile as tile
    from concourse import mybir
    from concourse.masks import make_identity
    from contextlib import ExitStack

    f32 = mybir.dt.float32
    bf16 = mybir.dt.bfloat16
    i32 = mybir.dt.int32
    AF = mybir.ActivationFunctionType
    OP = mybir.AluOpType
    AX = mybir.AxisListType

    nq = s_len // UB              # CRF q index = t // UB
    lanes = BL * nq               # CRF lane = b*nq + q  (b-major)
    ntile = (s_len * BL) // 128   # gather tiles of 128 tokens
    nblk = s_len // TB            # recurrence blocks

    nc = bacc.Bacc("TRN2", target_bir_lowering=False, debug=False,
                   enable_asserts=False, num_devices=n_devices)

    # ---- DRAM I/O -------------------------------------------------------
    d_embed = nc.dram_tensor("embed", [VOCAB + 1, EMB], f32, kind="ExternalInput").ap()
    d_idx = nc.dram_tensor("idx", [128, ntile], i32, kind="ExternalInput").ap()
    d_whh = nc.dram_tensor("whhT", [NLAYERS, 2, H2, 4 * H2], bf16, kind="ExternalInput").ap()
    d_wih0 = nc.dram_tensor("wih0T", [2, 2, 128, 4 * H2], bf16, kind="ExternalInput").ap()
    d_wih1 = nc.dram_tensor("wih1T", [2, 2, 128, 4 * H2], bf16, kind="ExternalInput").ap()
    d_wtag = nc.dram_tensor("wtagT", [2, 128, NTAGS], bf16, kind="ExternalInput").ap()
    d_mask = nc.dram_tensor("maskf", [s_len * BL], f32, kind="ExternalInput").ap()
    d_gsel = nc.dram_tensor("gsel", [lanes, UB, NTAGS], f32, kind="ExternalInput").ap()
    d_msel = nc.dram_tensor("msel", [lanes, UB], f32, kind="ExternalInput").ap()
    d_madd = nc.dram_tensor("madd", [lanes, UB, 16], f32, kind="ExternalInput").ap()
    d_trans = nc.dram_tensor("trans16", [16], f32, kind="ExternalInput").ap()
    d_start = nc.dram_tensor("startrep", [BL, NTAGS], f32, kind="ExternalInput").ap()
    d_end = nc.dram_tensor("endrep", [BL, 16], f32, kind="ExternalInput").ap()
    d_sel = nc.dram_tensor("sel2", [128, BL], f32, kind="ExternalInput").ap()

    d_logz = nc.dram_tensor("out_logz", [BL], f32, kind="ExternalOutput").ap()
    d_emit = nc.dram_tensor("out_emit", [BL], f32, kind="ExternalOutput").ap()

    with tile.TileContext(nc) as tc:
        with ExitStack() as ctx:
            consts = ctx.enter_context(tc.tile_pool(name="consts", bufs=1))
            big = ctx.enter_context(tc.tile_pool(name="big", bufs=1))
            work = ctx.enter_context(tc.tile_pool(name="work", bufs=6))
            gpool = ctx.enter_context(tc.tile_pool(name="gath", bufs=3))
            dscr = ctx.enter_context(
                tc.tile_pool(name="dscr", bufs=2, space=bass.MemorySpace.DRAM))
            _b = bass

            # ---- constants into SBUF ------------------------------------
            whh_sb = consts.tile([128, NLAYERS, 2, 4 * H2], bf16, tag="whh", name="whh")
            nc.sync.dma_start(out=whh_sb, in_=d_whh.rearrange("l d k m -> k l d m"))
            wih0_sb = consts.tile([128, 2, 2, 4 * H2], bf16, tag="wih0", name="wih0")
            nc.sync.dma_start(out=wih0_sb, in_=d_wih0.rearrange("d c k m -> k d c m"))
            wih1_sb = consts.tile([128, 2, 2, 4 * H2], bf16, tag="wih1", name="wih1")
            nc.sync.dma_start(out=wih1_sb, in_=d_wih1.rearrange("d c k m -> k d c m"))
            wtag_sb = consts.tile([128, 2, NTAGS], bf16, tag="wtag", name="wtag")
            nc.sync.dma_start(out=wtag_sb, in_=d_wtag.rearrange("c k m -> k c m"))
            idx_sb = consts.tile([128, ntile], i32, tag="idx", name="idx")
            nc.sync.dma_start(out=idx_sb, in_=d_idx)
            sel_sb = consts.tile([128, BL], f32, tag="sel", name="sel")
            nc.sync.dma_start(out=sel_sb, in_=d_sel)
            gsel_sb = consts.tile([lanes, UB, NTAGS], f32, tag="gsel", name="gsel")
            nc.sync.dma_start(out=gsel_sb, in_=d_gsel)
            msel_sb = consts.tile([lanes, UB], f32, tag="msel", name="msel")
            nc.sync.dma_start(out=msel_sb, in_=d_msel)
            madd_sb = consts.tile([lanes, UB, 16], f32, tag="madd", name="madd")
            nc.sync.dma_start(out=madd_sb, in_=d_madd)
            trans_sb = consts.tile([128, 16], f32, tag="trans", name="trans")
            nc.sync.dma_start(
                out=trans_sb,
                in_=_b.AP(tensor=d_trans.tensor, offset=0, ap=[[0, 128], [1, 16]]))
            start_sb = consts.tile([BL, NTAGS], f32, tag="start", name="start")
            nc.sync.dma_start(out=start_sb, in_=d_start)
            end_sb = consts.tile([BL, 16], f32, tag="end", name="end")
            nc.sync.dma_start(out=end_sb, in_=d_end)
            mask_sb = big.tile([128, s_len, BL], f32, tag="mask", name="mask")
            nc.sync.dma_start(
                out=mask_sb,
                in_=_b.AP(tensor=d_mask.tensor, offset=0,
                          ap=[[0, 128], [1, s_len * BL]]))
            ident = consts.tile([128, 128], f32, tag="ident", name="ident")
            make_identity(nc, ident)

            # ---- big state tiles ---------------------------------------
            x0 = big.tile([128, 2, s_len, BL], bf16, tag="x0", name="x0")
            h_layers = [big.tile([128, 2, s_len, BL], bf16, tag=f"H{l}", name=f"H{l}")
                        for l in range(NLAYERS)]
            em2 = big.tile([lanes, UB, NTAGS], f32, tag="em2", name="em2")

            # ---- phase A: embedding gather + transpose ------------------
            with tc.tile_pool(name="psA", bufs=4, space=bass.MemorySpace.PSUM) as psA:
                for T in range(ntile):
                    g_t = gpool.tile([128, EMB], f32, tag="gt", name="gt")
                    nc.gpsimd.indirect_dma_start(
                        out=g_t, out_offset=None,
                        in_=d_embed,
                        in_offset=bass.IndirectOffsetOnAxis(
                            ap=idx_sb[:, T:T + 1], axis=0),
                    )
                    for c in range(2):
                        tp = psA.tile([128, 128], f32, tag="tp", name="tp")
                        nc.tensor.transpose(tp, g_t[:, c * 128:(c + 1) * 128], ident)
                        nc.vector.tensor_copy(
                            x0[:, c, 16 * T:16 * (T + 1), :],
                            tp[:].rearrange("p (t b) -> p t b", b=BL))

            # ---- phases B-D: xproj + recurrence per layer ---------------
            # px tiles are 1 PSUM bank each ([128,4,TB,BL] f32, TB=16);
            # 3-deep rotation per direction so block n+2's x-projection can
            # stream into the PE interleaved with block n's recurrence
            # (fills the PE stall while the cell's ACT/DVE tail runs).
            with tc.tile_pool(name="psR", bufs=3, space=bass.MemorySpace.PSUM) as psR:
                for l in range(NLAYERS):
                    xin = x0 if l == 0 else h_layers[l - 1]
                    hout = h_layers[l]
                    wih = wih0_sb if l == 0 else wih1_sb
                    cz = [work.tile([128, 4, 2, BL], f32, tag=f"cz{d}",
                                    name=f"cz{l}{d}") for d in range(2)]

                    def blk_of(d, n):
                        return n if d == 0 else nblk - 1 - n

                    def emit_xproj(d, n, pt, g, kc):
                        blk = blk_of(d, n)
                        nc.tensor.matmul(
                            pt[:, g, :, :],
                            wih[:, d, kc, g * 128:(g + 1) * 128],
                            xin[:, kc, blk * TB:(blk + 1) * TB, :],
                            start=(kc == 0 and g == 0), stop=False,
                            skip_group_check=True)

                    pxq = {0: [], 1: []}
                    for n in range(min(2, nblk)):
                        for d in range(2):
                            pt = psR.tile([128, 4, TB, BL], f32, tag=f"px{d}",
                                          name=f"px{d}")
                            for g in range(4):
                                for kc in range(2):
                                    emit_xproj(d, n, pt, g, kc)
                            pxq[d].append(pt)

                    for blk_i in range(nblk):
                        px = [pxq[0].pop(0), pxq[1].pop(0)]
                        xq = []
                        if blk_i + 2 < nblk:
                            for d in range(2):
                                pt = psR.tile([128, 4, TB, BL], f32, tag=f"px{d}",
                                              name=f"px{d}")
                                pxq[d].append(pt)
                                for g in range(4):
                                    for kc in range(2):
                                        xq.append((d, blk_i + 2, pt, g, kc))
                        for j in range(TB):
                            sgs = [None, None]
                            czcs = [None, None]
                            for d in range(2):
                                blk = blk_of(d, blk_i)
                                t = blk * TB + (j if d == 0 else TB - 1 - j)
                                k = blk_i * TB + j
                                if k > 0:
                                    tprev = t - 1 if d == 0 else t + 1
                                    h_prev = hout[:, d, tprev, :]
                                    for g in range(4):
                                        nc.tensor.matmul(
                                            px[d][:, g, t % TB, :],
                                            whh_sb[:, l, d,
                                                   g * 128:(g + 1) * 128],
                                            h_prev,
                                            start=False,
                                            stop=(j == TB - 1 and g == 3),
                                            skip_group_check=True,
                                        )
                                # one x-projection matmul per dir-step fills
                                # the PE dependency stall with real work
                                if xq:
                                    xd, xn, xpt, xg, xkc = xq.pop(0)
                                    emit_xproj(xd, xn, xpt, xg, xkc)
                                pslice = px[d][:, :, t % TB, :]
                                czp = cz[d][:, (k - 1) % 4, :, :]
                                czc = cz[d][:, k % 4, :, :]
                                czcs[d] = czc
                                zslot = (czp if k > 0 else czc)[:, 1, :]
                                sg = work.tile([128, 4, BL], f32, tag=f"sg{d}",
                                               name=f"sg{d}")
                                sgs[d] = sg
                                if sigma_trick:
                                    nc.scalar.activation(sg, pslice, AF.Sigmoid)
                                    nc.vector.tensor_scalar(
                                        out=zslot, in0=sg[:, 3, :],
                                        scalar1=2.0, scalar2=-1.0,
                                        op0=OP.mult, op1=OP.add)
                                else:
                                    nc.scalar.activation(
                                        sg[:, 0:3, :], pslice[:, 0:3, :], AF.Sigmoid)
                                    nc.scalar.activation(
                                        zslot, pslice[:, 3, :], AF.Tanh)
                                cdst = czc[:, 0, :]
                                if k == 0:
                                    if d == 1:
                                        c2tmp = work.tile([128, BL], f32,
                                                          tag=f"c2{d}", name=f"c2{d}")
                                        nc.vector.tensor_tensor(
                                            out=c2tmp, in0=sg[:, 1, :], in1=zslot,
                                            op=OP.mult)
                                        nc.vector.tensor_tensor(
                                            out=cdst, in0=c2tmp,
                                            in1=mask_sb[:, t, :], op=OP.mult)
                                    else:
                                        nc.vector.tensor_tensor(
                                            out=cdst, in0=sg[:, 1, :], in1=zslot,
                                            op=OP.mult)
                                else:
                                    pp = work.tile([128, BL, 2], f32, tag=f"pp{d}",
                                                   name=f"pp{d}")
                                    nc.vector.tensor_tensor(
                                        out=pp,
                                        in0=sg[:, 0:2, :].transpose([0, 2, 1]),
                                        in1=czp.transpose([0, 2, 1]),
                                        op=OP.mult)
                                    if d == 1:
                                        c2tmp = work.tile([128, BL], f32,
                                                          tag=f"c2{d}", name=f"c2{d}")
                                        nc.vector.tensor_reduce(
                                            out=c2tmp, in_=pp, axis=AX.X, op=OP.add)
                                        nc.vector.tensor_tensor(
                                            out=cdst, in0=c2tmp,
                                            in1=mask_sb[:, t, :], op=OP.mult)
                                    else:
                                        nc.vector.tensor_reduce(
                                            out=cdst, in_=pp, axis=AX.X, op=OP.add)
                            for d in range(2):
                                blk = blk_of(d, blk_i)
                                t = blk * TB + (j if d == 0 else TB - 1 - j)
                                sg = sgs[d]
                                cdst = czcs[d][:, 0, :]
                                sc = work.tile([128, BL], f32, tag=f"sc{d}",
                                               name=f"sc{d}")
                                if sigma_trick:
                                    nc.scalar.activation(sc, cdst, AF.Sigmoid,
                                                         scale=2.0)
                                    z2 = work.tile([128, BL], f32, tag=f"z2{d}",
                                                   name=f"z2{d}")
                                    # chain tail on GPSIMD: frees DVE cycles
                                    nc.gpsimd.tensor_scalar(
                                        out=z2, in0=sc, scalar1=2.0, scalar2=-1.0,
                                        op0=OP.mult, op1=OP.add)
                                else:
                                    nc.scalar.activation(sc, cdst, AF.Tanh)
                                    z2 = sc
                                nc.gpsimd.tensor_tensor(
                                    out=hout[:, d, t, :],
                                    in0=sg[:, 2, :], in1=z2,
                                    op=OP.mult)

            # ---- phase E/F: tag projection + emission dot ---------------
            tpool = ctx.enter_context(tc.tile_pool(name="tree", bufs=2))
            nchunk = (s_len * BL) // 128
            emT = big.tile([128, nchunk, NTAGS], f32, tag="emT", name="emT")
            with tc.tile_pool(name="psE", bufs=4, space=bass.MemorySpace.PSUM) as psE:
                h1 = h_layers[NLAYERS - 1]
                for ch in range(nchunk):
                    pe = psE.tile([128, NTAGS], f32, tag="pe", name="pe")
                    for kc in range(2):
                        lhs = h1[:, kc, 16 * ch:16 * (ch + 1), :].rearrange(
                            "p t b -> p (t b)")
                        nc.tensor.matmul(pe, lhs, wtag_sb[:, kc, :],
                                         start=(kc == 0), stop=(kc == 1))
                    nc.vector.tensor_copy(emT[:, ch, :], pe)
                # permute token rows (t*8+b) -> CRF lanes (b*nq+q, u) via DRAM
                demT = dscr.tile([128, nchunk, NTAGS], f32, tag="demT", name="demT")
                nc.sync.dma_start(out=demT, in_=emT)
                # dram addr(p=u0*BL+b, c=2q+u1, j) = p*nchunk*NTAGS + c*NTAGS + j
                dt_ap = demT[:]
                nc.sync.dma_start(
                    out=em2,
                    in_=_b.AP(tensor=dt_ap.tensor, offset=dt_ap.offset,
                              ap=[[nchunk * NTAGS, BL],         # b
                                  [2 * NTAGS, nq],              # q
                                  [NTAGS, 2],                   # u1 = u//16
                                  [BL * nchunk * NTAGS, 16],    # u0 = u%16
                                  [1, NTAGS]]))                 # j

                prod = big.tile([lanes, UB, NTAGS], f32, tag="prod", name="prod")
                nc.vector.tensor_tensor(out=prod, in0=em2, in1=gsel_sb, op=OP.mult)
                rsum = work.tile([lanes, 1], f32, tag="rsum", name="rsum")
                nc.vector.tensor_reduce(out=rsum, in_=prod, axis=AX.XY, op=OP.add)
                pemit = psE.tile([BL, 1], f32, tag="pemit", name="pemit")
                nc.tensor.matmul(pemit, sel_sb[:lanes, :], rsum,
                                 start=True, stop=True)
                emit_sb = work.tile([BL, 1], f32, tag="emit", name="emit")
                nc.vector.tensor_copy(emit_sb, pemit)
                nc.sync.dma_start(out=d_emit, in_=emit_sb)

            # ---- phase G: CRF partition via log-semiring tree -----------
            mten = big.tile([lanes, UB, 16], f32, tag="M", name="M")
            nc.vector.tensor_tensor(
                out=mten[:].rearrange("p u (i j) -> p u i j", i=4),
                in0=trans_sb[:lanes, :].rearrange("p (i j) -> p i j", i=4)
                    .unsqueeze(1).broadcast_to([lanes, UB, NTAGS, NTAGS]),
                in1=em2[:].unsqueeze(2)
                    .broadcast_to([lanes, UB, NTAGS, NTAGS]),
                op=OP.add)
            m2t = big.tile([lanes, UB, 16], f32, tag="M2", name="M2")
            nc.vector.tensor_tensor(
                out=m2t, in0=mten,
                in1=msel_sb[:].unsqueeze(2).broadcast_to([lanes, UB, 16]),
                op=OP.mult)
            cur = big.tile([lanes, UB, 16], f32, tag="M3", name="M3")
            nc.vector.tensor_tensor(out=cur, in0=m2t, in1=madd_sb, op=OP.add)
            cur = cur[:]

            def combine(a_ap, b_ap, npart, nu2, out_ap):
                # a,b: [npart, nu2, 16]; out[i,k] = lse_j a[i,j] + b[j,k]
                av = a_ap.rearrange("p u (i j) -> p u i j", i=4)
                bv = b_ap.rearrange("p u (j k) -> p u j k", j=4) \
                    .transpose([0, 1, 3, 2])  # [p, u, k, j]
                xt = tpool.tile([npart, nu2, 4, 4, 4], f32, tag="X", name="X")
                for i in range(4):
                    nc.vector.tensor_tensor(
                        out=xt[:, :, i, :, :],
                        in0=av[:, :, i, :].unsqueeze(2)
                            .broadcast_to([npart, nu2, 4, 4]),
                        in1=bv, op=OP.add)
                mx = tpool.tile([npart, nu2, 4, 4], f32, tag="mx", name="mx")
                nc.vector.tensor_reduce(
                    out=mx[:].rearrange("p u i k -> p (u i k)"),
                    in_=xt[:].rearrange("p u i k j -> p (u i k) j"),
                    axis=AX.X, op=OP.max)
                xs = tpool.tile([npart, nu2, 4, 4, 4], f32, tag="XS", name="XS")
                for i in range(4):
                    nc.vector.tensor_tensor(
                        out=xs[:, :, i, :, :], in0=xt[:, :, i, :, :],
                        in1=mx[:, :, i, :].unsqueeze(3)
                            .broadcast_to([npart, nu2, 4, 4]),
                        op=OP.subtract)
                ex = tpool.tile([npart, nu2, 4, 4, 4], f32, tag="EX", name="EX")
                nc.scalar.activation(
                    ex[:].rearrange("p u i k j -> p (u i k j)"),
                    xs[:].rearrange("p u i k j -> p (u i k j)"), AF.Exp)
                sm = tpool.tile([npart, nu2, 4, 4], f32, tag="sm", name="sm")
                nc.vector.tensor_reduce(
                    out=sm[:].rearrange("p u i k -> p (u i k)"),
                    in_=ex[:].rearrange("p u i k j -> p (u i k) j"),
                    axis=AX.X, op=OP.add)
                ln = tpool.tile([npart, nu2, 4, 4], f32, tag="ln", name="ln")
                nc.scalar.activation(
                    ln[:].rearrange("p u i k -> p (u i k)"),
                    sm[:].rearrange("p u i k -> p (u i k)"), AF.Ln)
                nc.vector.tensor_tensor(
                    out=out_ap,
                    in0=ln[:].rearrange("p u i k -> p u (i k)"),
                    in1=mx[:].rearrange("p u i k -> p u (i k)"), op=OP.add)

            nu = UB
            while nu > 1:
                nxt = tpool.tile([lanes, nu // 2, 16], f32, tag="cur", name="cur")
                combine(cur[:, 0::2, :], cur[:, 1::2, :], lanes, nu // 2, nxt[:])
                cur = nxt[:]
                nu //= 2
            nl = lanes
            while nl > BL:
                half = nl // 2
                dsc = dscr.tile([nl, 16], f32, tag="dsc", name="dsc")
                nc.sync.dma_start(out=dsc, in_=cur[:, 0, :])
                a_t = tpool.tile([half, 1, 16], f32, tag="Ac", name="Ac")
                b_t = tpool.tile([half, 1, 16], f32, tag="Bc", name="Bc")
                nc.sync.dma_start(out=a_t[:, 0, :], in_=dsc[0::2, :])
                nc.sync.dma_start(out=b_t[:, 0, :], in_=dsc[1::2, :])
                nxt = tpool.tile([half, 1, 16], f32, tag="cur", name="cur")
                combine(a_t[:], b_t[:], half, 1, nxt[:])
                cur = nxt[:]
                nl = half

            dsc2 = dscr.tile([lanes, NTAGS], f32, tag="dsc2", name="dsc2")
            nc.sync.dma_start(out=dsc2, in_=em2[:, 0, :])
            em0 = tpool.tile([BL, NTAGS], f32, tag="em0", name="em0")
            nc.sync.dma_start(out=em0, in_=dsc2[0::nq, :])
            a0 = tpool.tile([BL, NTAGS], f32, tag="a0", name="a0")
            nc.vector.tensor_tensor(out=a0, in0=em0, in1=start_sb, op=OP.add)
            y1 = tpool.tile([BL, 16], f32, tag="y1", name="y1")
            nc.vector.tensor_tensor(
                out=y1[:].rearrange("p (i k) -> p i k", i=4),
                in0=cur.rearrange("p u (i k) -> p (u i) k", i=4),
                in1=a0[:].unsqueeze(2).broadcast_to([BL, NTAGS, NTAGS]),
                op=OP.add)
            y2 = tpool.tile([BL, 16], f32, tag="y2", name="y2")
            nc.vector.tensor_tensor(out=y2, in0=y1, in1=end_sb, op=OP.add)
            mxf = tpool.tile([BL, 1], f32, tag="mxf", name="mxf")
            nc.vector.tensor_reduce(out=mxf, in_=y2, axis=AX.X, op=OP.max)
            yd = tpool.tile([BL, 16], f32, tag="yd", name="yd")
            nc.vector.tensor_scalar(out=yd, in0=y2, scalar1=mxf[:], scalar2=None,
                                    op0=OP.subtract)
            ye = tpool.tile([BL, 16], f32, tag="ye", name="ye")
            sme = tpool.tile([BL, 1], f32, tag="sme", name="sme")
            nc.scalar.activation(ye, yd, AF.Exp, accum_out=sme[:])
            lns = tpool.tile([BL, 1], f32, tag="lns", name="lns")
            nc.scalar.activation(lns, sme, AF.Ln)
            lz = tpool.tile([BL, 1], f32, tag="lz", name="lz")
            nc.vector.tensor_tensor(out=lz, in0=lns, in1=mxf, op=OP.add)
            nc.sync.dma_start(out=d_logz, in_=lz)

    nc.compile()
    return nc


# --------------------------------------------------------------------------
# Host preparation
# --------------------------------------------------------------------------

def prep_core_inputs(core, sentence, tags, mask_f, length, embed_full,
                     w_ih, w_hh, w_tag, start_trans, end_trans, trans,
                     s_len=S, sigma_trick=True):
    nq = s_len // UB
    lanes = BL * nq
    ntile = (s_len * BL) // 128
    bsl = slice(core * BL, (core + 1) * BL)
    sent = np.asarray(sentence)[bsl, :s_len]
    tg = np.asarray(tags)[bsl, :s_len]
    mf = np.asarray(mask_f)[bsl, :s_len].astype(np.float32)

    # token gather index: tile T covers t in [16T,16T+16); p = (t%16)*8 + b
    tt = 16 * np.arange(ntile)[None, :] + (np.arange(128) // BL)[:, None]
    bb = (np.arange(128) % BL)[:, None] + np.zeros((1, ntile), np.int64)
    idx = sent[bb, tt].astype(np.int32)

    # gate order (f, i, o, g); reference splits gates as (i, f, g, o)
    perm = np.concatenate([np.arange(H2, 2 * H2),      # f
                           np.arange(0, H2),           # i
                           np.arange(3 * H2, 4 * H2),  # o
                           np.arange(2 * H2, 3 * H2)]) # g
    gscale = np.ones((4 * H2, 1), np.float32)
    if sigma_trick:
        gscale[3 * H2:] = 2.0

    def pack_w(w):  # w [4H2, K] -> [K, 4H2] reordered (+g-scaled)
        wr = (np.asarray(w, np.float32)[perm, :] * gscale)
        return np.ascontiguousarray(wr.T)

    whhT = np.stack([np.stack([pack_w(w_hh[l, d]) for d in range(2)])
                     for l in range(NLAYERS)])
    wih0T = np.stack([
        np.stack([pack_w(w_ih[0, d])[kc * 128:(kc + 1) * 128] for kc in range(2)])
        for d in range(2)])
    wih1T = np.stack([
        np.stack([pack_w(w_ih[1, d])[kc * 128:(kc + 1) * 128] for kc in range(2)])
        for d in range(2)])
    wtagT = np.ascontiguousarray(np.asarray(w_tag, np.float32).T)
    wtagT = np.stack([wtagT[:128], wtagT[128:]])

    maskf = np.ascontiguousarray(mf.T).reshape(-1)

    tarr = np.arange(s_len)
    qv, uv = tarr // UB, tarr % UB
    gsel = np.zeros((lanes, UB, NTAGS), np.float32)
    msel = np.zeros((lanes, UB), np.float32)
    madd = np.zeros((lanes, UB, 16), np.float32)
    offd = (1.0 - np.eye(NTAGS, dtype=np.float32)).reshape(16)
    for b in range(BL):
        for t in range(s_len):
            lane, u = b * nq + qv[t], uv[t]
            coef = 1.0 if t == 0 else float(mf[b, t])
            gsel[lane, u, int(tg[b, t])] = coef
            valid = (t >= 1) and mf[b, t] > 0
            msel[lane, u] = 1.0 if valid else 0.0
            if not valid:
                madd[lane, u] = NEG * offd

    trans16 = np.ascontiguousarray(np.asarray(trans, np.float32).reshape(16))
    startrep = np.broadcast_to(
        np.asarray(start_trans, np.float32), (BL, NTAGS)).copy()
    endrep = np.broadcast_to(np.asarray(end_trans, np.float32)[None, None, :],
                             (BL, NTAGS, NTAGS)).reshape(BL, 16).copy()
    sel2 = np.zeros((128, BL), np.float32)
    for p in range(lanes):
        sel2[p, p // nq] = 1.0

    import ml_dtypes
    bf = ml_dtypes.bfloat16
    return {
        "embed": embed_full,
        "idx": np.ascontiguousarray(idx),
        "whhT": np.ascontiguousarray(whhT).astype(bf),
        "wih0T": np.ascontiguousarray(wih0T).astype(bf),
        "wih1T": np.ascontiguousarray(wih1T).astype(bf),
        "wtagT": np.ascontiguousarray(wtagT).astype(bf),
        "maskf": maskf,
        "gsel": gsel,
        "msel": msel,
        "madd": madd,
        "trans16": trans16,
        "startrep": startrep,
        "endrep": endrep,
        "sel2": sel2,
    }


def host_trans_score(tags, mask_f, length, start_trans, end_trans, trans):
    tags = np.asarray(tags)
    Bn = tags.shape[0]
    ar = np.arange(Bn)
    sc = np.asarray(start_trans)[tags[:, 0]].astype(np.float64)
    tr = np.asarray(trans)[tags[:, :-1], tags[:, 1:]]
    sc = sc + np.sum(tr * np.asarray(mask_f)[:, 1:], axis=1)
    last = tags[ar, np.asarray(length) - 1]
    sc = sc + np.asarray(end_trans)[last]
    return sc


# --------------------------------------------------------------------------
# Public entry
# --------------------------------------------------------------------------

def kernel(**inputs):
    return _run(inputs, trace=False)[0]


def _run(inputs, trace=False):
    loss, res = _run_impl(trace=trace, **inputs)
    return loss, res


def _run_impl(sentence, tags, mask, length, embed, w_ih, w_hh, b_ih, b_hh,
              w_tag, b_tag, start_trans, end_trans, trans, trace=False):
    from concourse import bass_utils

    sentence = np.asarray(sentence).astype(np.int64)
    tags = np.asarray(tags).astype(np.int64)
    mask_f = np.asarray(mask).astype(np.float32)
    length = np.asarray(length).astype(np.int64)
    embed = np.ascontiguousarray(np.asarray(embed, np.float32))
    w_ih = np.asarray(w_ih, np.float32)
    w_hh = np.asarray(w_hh, np.float32)
    w_tag = np.asarray(w_tag, np.float32)
    start_trans = np.asarray(start_trans, np.float32)
    end_trans = np.asarray(end_trans, np.float32)
    trans = np.asarray(trans, np.float32)

    assert np.all(np.asarray(b_ih) == 0) and np.all(np.asarray(b_hh) == 0) \
        and np.all(np.asarray(b_tag) == 0), "kernel assumes zero biases"

    key = ("prog", S, True)
    if key not in _BUILD_CACHE:
        _BUILD_CACHE[key] = build_program(S, sigma_trick=True)
    nc = _BUILD_CACHE[key]

    in_maps = [prep_core_inputs(core, sentence, tags, mask_f, length, embed,
                                w_ih, w_hh, w_tag, start_trans, end_trans, trans)
               for core in range(NCORES)]

    res = bass_utils.run_bass_kernel_spmd(nc, in_maps, core_ids=list(range(NCORES)),
                                          trace=trace)

    logz = np.concatenate([r["out_logz"] for r in res.results]).astype(np.float64)
    emit = np.concatenate([r["out_emit"] for r in res.results]).astype(np.float64)
    tsc = host_trans_score(tags, mask_f, length, start_trans, end_trans, trans)
    llh = (tsc + emit) - logz
    return np.float32(-np.mean(llh)), res



# revision 5
# speedup vs baseline: 1.2810x; 1.2810x over previous
# BiLSTM-CRF negative log-likelihood on 8 Trainium2 NeuronCores.
# Self-contained: host prep + Bass/Tile device program + unshard.
#
# Sharding: data-parallel over batch. 64 sequences -> 8 cores x 8 seqs.
# Per core: embedding gather -> 2-layer BiLSTM -> tag projection ->
# CRF partition (log-semiring tree reduction) + gold emission dot.
# Host: gold transition score (pure index math on host-visible inputs),
# final llh assembly and mean.

import numpy as np

VOCAB = 50000
EMB = 256
HID = 256
H2 = 128
NLAYERS = 2
NTAGS = 4
B = 64
S = 512
NCORES = 8
BL = B // NCORES          # sequences per core
TB = 16                   # timesteps per x-projection PSUM block
UB = 32                   # CRF tree: timesteps per lane (q = t // UB)
NEG = -1.0e9

_BUILD_CACHE = {}


# --------------------------------------------------------------------------
# Device program
# --------------------------------------------------------------------------

def build_program(s_len=S, sigma_trick=True, n_devices=NCORES):
    """Builds the per-core Bass program (identical on all cores)."""
    import concourse.bacc as bacc
    import concourse.bass as bass
    import concourse.tile as tile
    from concourse import mybir
    from concourse.masks import make_identity
    from contextlib import ExitStack

    f32 = mybir.dt.float32
    bf16 = mybir.dt.bfloat16
    i32 = mybir.dt.int32
    AF = mybir.ActivationFunctionType
    OP = mybir.AluOpType
    AX = mybir.AxisListType

    nq = s_len // UB              # CRF q index = t // UB
    lanes = BL * nq               # CRF lane = b*nq + q  (b-major)
    ntile = (s_len * BL) // 128   # gather tiles of 128 tokens
    nblk = s_len // TB            # recurrence blocks

    nc = bacc.Bacc("TRN2", target_bir_lowering=False, debug=False,
                   enable_asserts=False, num_devices=n_devices)

    # ---- DRAM I/O -------------------------------------------------------
    d_embed = nc.dram_tensor("embed", [VOCAB + 1, EMB], f32, kind="ExternalInput").ap()
    d_idx = nc.dram_tensor("idx", [128, ntile], i32, kind="ExternalInput").ap()
    d_whh = nc.dram_tensor("whhT", [NLAYERS, 2, H2, 4 * H2], bf16, kind="ExternalInput").ap()
    d_wih0 = nc.dram_tensor("wih0T", [2, 2, 128, 4 * H2], bf16, kind="ExternalInput").ap()
    d_wih1 = nc.dram_tensor("wih1T", [2, 2, 128, 4 * H2], bf16, kind="ExternalInput").ap()
    d_wtag = nc.dram_tensor("wtagT", [2, 128, NTAGS], bf16, kind="ExternalInput").ap()
    d_mask = nc.dram_tensor("maskf", [s_len * BL], f32, kind="ExternalInput").ap()
    d_gsel = nc.dram_tensor("gsel", [lanes, UB, NTAGS], f32, kind="ExternalInput").ap()
    d_msel = nc.dram_tensor("msel", [lanes, UB], f32, kind="ExternalInput").ap()
    d_madd = nc.dram_tensor("madd", [lanes, UB, 16], f32, kind="ExternalInput").ap()
    d_trans = nc.dram_tensor("trans16", [16], f32, kind="ExternalInput").ap()
    d_start = nc.dram_tensor("startrep", [BL, NTAGS], f32, kind="ExternalInput").ap()
    d_end = nc.dram_tensor("endrep", [BL, 16], f32, kind="ExternalInput").ap()
    d_sel = nc.dram_tensor("sel2", [128, BL], f32, kind="ExternalInput").ap()

    d_logz = nc.dram_tensor("out_logz", [BL], f32, kind="ExternalOutput").ap()
    d_emit = nc.dram_tensor("out_emit", [BL], f32, kind="ExternalOutput").ap()

    with tile.TileContext(nc) as tc:
        with ExitStack() as ctx:
            consts = ctx.enter_context(tc.tile_pool(name="consts", bufs=1))
            big = ctx.enter_context(tc.tile_pool(name="big", bufs=1))
            work = ctx.enter_context(tc.tile_pool(name="work", bufs=6))
            gpool = ctx.enter_context(tc.tile_pool(name="gath", bufs=3))
            dscr = ctx.enter_context(
                tc.tile_pool(name="dscr", bufs=2, space=bass.MemorySpace.DRAM))
            _b = bass

            # ---- constants into SBUF ------------------------------------
            whh_sb = consts.tile([128, NLAYERS, 2, 4 * H2], bf16, tag="whh", name="whh")
            nc.sync.dma_start(out=whh_sb, in_=d_whh.rearrange("l d k m -> k l d m"))
            wih0_sb = consts.tile([128, 2, 2, 4 * H2], bf16, tag="wih0", name="wih0")
            nc.sync.dma_start(out=wih0_sb, in_=d_wih0.rearrange("d c k m -> k d c m"))
            wih1_sb = consts.tile([128, 2, 2, 4 * H2], bf16, tag="wih1", name="wih1")
            nc.sync.dma_start(out=wih1_sb, in_=d_wih1.rearrange("d c k m -> k d c m"))
            wtag_sb = consts.tile([128, 2, NTAGS], bf16, tag="wtag", name="wtag")
            nc.sync.dma_start(out=wtag_sb, in_=d_wtag.rearrange("c k m -> k c m"))
            idx_sb = consts.tile([128, ntile], i32, tag="idx", name="idx")
            nc.sync.dma_start(out=idx_sb, in_=d_idx)
            sel_sb = consts.tile([128, BL], f32, tag="sel", name="sel")
            nc.sync.dma_start(out=sel_sb, in_=d_sel)
            gsel_sb = consts.tile([lanes, UB, NTAGS], f32, tag="gsel", name="gsel")
            nc.sync.dma_start(out=gsel_sb, in_=d_gsel)
            msel_sb = consts.tile([lanes, UB], f32, tag="msel", name="msel")
            nc.sync.dma_start(out=msel_sb, in_=d_msel)
            madd_sb = consts.tile([lanes, UB, 16], f32, tag="madd", name="madd")
            nc.sync.dma_start(out=madd_sb, in_=d_madd)
            trans_sb = consts.tile([128, 16], f32, tag="trans", name="trans")
            nc.sync.dma_start(
                out=trans_sb,
                in_=_b.AP(tensor=d_trans.tensor, offset=0, ap=[[0, 128], [1, 16]]))
            start_sb = consts.tile([BL, NTAGS], f32, tag="start", name="start")
            nc.sync.dma_start(out=start_sb, in_=d_start)
            end_sb = consts.tile([BL, 16], f32, tag="end", name="end")
            nc.sync.dma_start(out=end_sb, in_=d_end)
            mask_sb = big.tile([128, s_len, BL], f32, tag="mask", name="mask")
            nc.sync.dma_start(
                out=mask_sb,
                in_=_b.AP(tensor=d_mask.tensor, offset=0,
                          ap=[[0, 128], [1, s_len * BL]]))
            ident = consts.tile([128, 128], f32, tag="ident", name="ident")
            make_identity(nc, ident)

            # ---- big state tiles ---------------------------------------
            x0 = big.tile([128, 2, s_len, BL], bf16, tag="x0", name="x0")
            h_layers = [big.tile([128, 2, s_len, BL], bf16, tag=f"H{l}", name=f"H{l}")
                        for l in range(NLAYERS)]
            em2 = big.tile([lanes, UB, NTAGS], f32, tag="em2", name="em2")

            # ---- phase A: embedding gather + transpose ------------------
            with tc.tile_pool(name="psA", bufs=4, space=bass.MemorySpace.PSUM) as psA:
                for T in range(ntile):
                    g_t = gpool.tile([128, EMB], f32, tag="gt", name="gt")
                    nc.gpsimd.indirect_dma_start(
                        out=g_t, out_offset=None,
                        in_=d_embed,
                        in_offset=bass.IndirectOffsetOnAxis(
                            ap=idx_sb[:, T:T + 1], axis=0),
                    )
                    for c in range(2):
                        tp = psA.tile([128, 128], f32, tag="tp", name="tp")
                        nc.tensor.transpose(tp, g_t[:, c * 128:(c + 1) * 128], ident)
                        nc.vector.tensor_copy(
                            x0[:, c, 16 * T:16 * (T + 1), :],
                            tp[:].rearrange("p (t b) -> p t b", b=BL))

            # ---- phases B-D: xproj + recurrence per layer ---------------
            # px tiles are 1 PSUM bank each ([128,4,TB,BL] f32, TB=16);
            # 3-deep rotation per direction so block n+2's x-projection can
            # stream into the PE interleaved with block n's recurrence
            # (fills the PE stall while the cell's ACT/DVE tail runs).
            #
            # Cell math tracks ct = c/2 so the update is a plain sum of four
            # products (no extra affine op for tanh(g)):
            #   ct2 = sf*ct1 + si*sg - si/2
            # with sg = sigmoid(2*gtilde) (g-weights prescaled by 2) laid out
            # as (si,si,sf,sf) (.) (sg, -1/2, ct1, 0) over one sg ring tile:
            # slots 0..3 = sigmoid(gates i,f,o,g), 4 = ct1, 5 = -1/2, 6 = 0.
            # tanh(c) = 2*sigmoid(4*ct)-1 and h~ = h/2 = (sc-1/2)*so in one
            # scalar_tensor_tensor; the missing 2x lives in the host-side
            # prescale of whh/wih1/wtag.
            RING = 4
            cellp = ctx.enter_context(tc.tile_pool(name="cell", bufs=1))
            ring = [[cellp.tile([128, 7, BL], f32, tag=f"ring{d}{r}",
                                name=f"ring{d}{r}") for r in range(RING)]
                    for d in range(2)]
            for d in range(2):
                for r in range(RING):
                    nc.vector.memset(ring[d][r][:, 5, :], -0.5)
                    nc.vector.memset(ring[d][r][:, 6, :], 0.0)
            with tc.tile_pool(name="psR", bufs=3, space=bass.MemorySpace.PSUM) as psR:
                for l in range(NLAYERS):
                    xin = x0 if l == 0 else h_layers[l - 1]
                    hout = h_layers[l]
                    wih = wih0_sb if l == 0 else wih1_sb
                    for d in range(2):
                        for r in range(RING):
                            nc.vector.memset(ring[d][r][:, 4, :], 0.0)

                    def blk_of(d, n):
                        return n if d == 0 else nblk - 1 - n

                    def emit_xproj(d, n, pt, g, kc):
                        blk = blk_of(d, n)
                        nc.tensor.matmul(
                            pt[:, g, :, :],
                            wih[:, d, kc, g * 128:(g + 1) * 128],
                            xin[:, kc, blk * TB:(blk + 1) * TB, :],
                            start=(kc == 0 and g == 0), stop=False,
                            skip_group_check=True)

                    pxq = {0: [], 1: []}
                    for n in range(min(2, nblk)):
                        for d in range(2):
                            pt = psR.tile([128, 4, TB, BL], f32, tag=f"px{d}",
                                          name=f"px{d}")
                            for g in range(4):
                                for kc in range(2):
                                    emit_xproj(d, n, pt, g, kc)
                            pxq[d].append(pt)

                    for blk_i in range(nblk):
                        px = [pxq[0].pop(0), pxq[1].pop(0)]
                        xq = []
                        if blk_i + 2 < nblk:
                            for d in range(2):
                                pt = psR.tile([128, 4, TB, BL], f32, tag=f"px{d}",
                                              name=f"px{d}")
                                pxq[d].append(pt)
                                for g in range(4):
                                    for kc in range(2):
                                        xq.append((d, blk_i + 2, pt, g, kc))
                        for j in range(TB):
                            sgs = [None, None]
                            nxts = [None, None]
                            for d in range(2):
                                blk = blk_of(d, blk_i)
                                t = blk * TB + (j if d == 0 else TB - 1 - j)
                                k = blk_i * TB + j
                                if k > 0:
                                    tprev = t - 1 if d == 0 else t + 1
                                    h_prev = hout[:, d, tprev, :]
                                    for g in range(4):
                                        nc.tensor.matmul(
                                            px[d][:, g, t % TB, :],
                                            whh_sb[:, l, d,
                                                   g * 128:(g + 1) * 128],
                                            h_prev,
                                            start=False,
                                            stop=(j == TB - 1 and g == 3),
                                            skip_group_check=True,
                                        )
                                # one x-projection matmul per dir-step fills
                                # the PE dependency stall with real work
                                if xq:
                                    xd, xn, xpt, xg, xkc = xq.pop(0)
                                    emit_xproj(xd, xn, xpt, xg, xkc)
                                pslice = px[d][:, :, t % TB, :]
                                sg = ring[d][k % RING]
                                nxt = ring[d][(k + 1) % RING]
                                sgs[d] = sg
                                nxts[d] = nxt
                                nc.scalar.activation(sg[:, 0:4, :], pslice,
                                                     AF.Sigmoid)
                                # ct2 = si*sg + si*(-1/2) + sf*ct1 + sf*0
                                prod = work.tile([128, BL, 4], f32,
                                                 tag=f"prod{d}", name=f"prod{d}")
                                in0 = sg[:, 0:2, :].unsqueeze(2) \
                                    .broadcast_to([128, 2, 2, BL])
                                in1 = sg[:, 3:7, :].rearrange(
                                    "p (m0 m1) b -> p m1 m0 b", m0=2)
                                outv = prod[:].rearrange(
                                    "p b (m1 m0) -> p m1 m0 b", m1=2)
                                nc.vector.tensor_tensor(
                                    out=outv, in0=in0, in1=in1, op=OP.mult)
                                if d == 1:
                                    c2tmp = work.tile([128, BL], f32,
                                                      tag=f"c2{d}", name=f"c2{d}")
                                    nc.vector.tensor_reduce(
                                        out=c2tmp, in_=prod[:], axis=AX.X,
                                        op=OP.add)
                                    nc.vector.tensor_tensor(
                                        out=nxt[:, 4, :], in0=c2tmp,
                                        in1=mask_sb[:, t, :], op=OP.mult)
                                else:
                                    nc.vector.tensor_reduce(
                                        out=nxt[:, 4, :], in_=prod[:],
                                        axis=AX.X, op=OP.add)
                            for d in range(2):
                                blk = blk_of(d, blk_i)
                                t = blk * TB + (j if d == 0 else TB - 1 - j)
                                sg = sgs[d]
                                # sc = sigmoid(4*ct2) = (tanh(c2)+1)/2
                                sc = work.tile([128, BL], f32, tag=f"sc{d}",
                                               name=f"sc{d}")
                                nc.scalar.activation(sc, nxts[d][:, 4, :],
                                                     AF.Sigmoid, scale=4.0)
                                # h~ = h/2 = (sc - 1/2) * so
                                nc.vector.scalar_tensor_tensor(
                                    out=hout[:, d, t, :],
                                    in0=sc, scalar=-0.5, in1=sg[:, 2, :],
                                    op0=OP.add, op1=OP.mult)

            # ---- phase E/F: tag projection + emission dot ---------------
            tpool = ctx.enter_context(tc.tile_pool(name="tree", bufs=2))
            nchunk = (s_len * BL) // 128
            emT = big.tile([128, nchunk, NTAGS], f32, tag="emT", name="emT")
            with tc.tile_pool(name="psE", bufs=4, space=bass.MemorySpace.PSUM) as psE:
                h1 = h_layers[NLAYERS - 1]
                for ch in range(nchunk):
                    pe = psE.tile([128, NTAGS], f32, tag="pe", name="pe")
                    for kc in range(2):
                        lhs = h1[:, kc, 16 * ch:16 * (ch + 1), :].rearrange(
                            "p t b -> p (t b)")
                        nc.tensor.matmul(pe, lhs, wtag_sb[:, kc, :],
                                         start=(kc == 0), stop=(kc == 1))
                    nc.vector.tensor_copy(emT[:, ch, :], pe)
                # permute token rows (t*8+b) -> CRF lanes (b*nq+q, u) via DRAM
                demT = dscr.tile([128, nchunk, NTAGS], f32, tag="demT", name="demT")
                nc.sync.dma_start(out=demT, in_=emT)
                # dram addr(p=u0*BL+b, c=2q+u1, j) = p*nchunk*NTAGS + c*NTAGS + j
                dt_ap = demT[:]
                nc.sync.dma_start(
                    out=em2,
                    in_=_b.AP(tensor=dt_ap.tensor, offset=dt_ap.offset,
                              ap=[[nchunk * NTAGS, BL],         # b
                                  [2 * NTAGS, nq],              # q
                                  [NTAGS, 2],                   # u1 = u//16
                                  [BL * nchunk * NTAGS, 16],    # u0 = u%16
                                  [1, NTAGS]]))                 # j

                prod = big.tile([lanes, UB, NTAGS], f32, tag="prod", name="prod")
                nc.vector.tensor_tensor(out=prod, in0=em2, in1=gsel_sb, op=OP.mult)
                rsum = work.tile([lanes, 1], f32, tag="rsum", name="rsum")
                nc.vector.tensor_reduce(out=rsum, in_=prod, axis=AX.XY, op=OP.add)
                pemit = psE.tile([BL, 1], f32, tag="pemit", name="pemit")
                nc.tensor.matmul(pemit, sel_sb[:lanes, :], rsum,
                                 start=True, stop=True)
                emit_sb = work.tile([BL, 1], f32, tag="emit", name="emit")
                nc.vector.tensor_copy(emit_sb, pemit)
                nc.sync.dma_start(out=d_emit, in_=emit_sb)

            # ---- phase G: CRF partition via log-semiring tree -----------
            mten = big.tile([lanes, UB, 16], f32, tag="M", name="M")
            nc.vector.tensor_tensor(
                out=mten[:].rearrange("p u (i j) -> p u i j", i=4),
                in0=trans_sb[:lanes, :].rearrange("p (i j) -> p i j", i=4)
                    .unsqueeze(1).broadcast_to([lanes, UB, NTAGS, NTAGS]),
                in1=em2[:].unsqueeze(2)
                    .broadcast_to([lanes, UB, NTAGS, NTAGS]),
                op=OP.add)
            m2t = big.tile([lanes, UB, 16], f32, tag="M2", name="M2")
            nc.vector.tensor_tensor(
                out=m2t, in0=mten,
                in1=msel_sb[:].unsqueeze(2).broadcast_to([lanes, UB, 16]),
                op=OP.mult)
            cur = big.tile([lanes, UB, 16], f32, tag="M3", name="M3")
            nc.vector.tensor_tensor(out=cur, in0=m2t, in1=madd_sb, op=OP.add)
            cur = cur[:]

            def combine(a_ap, b_ap, npart, nu2, out_ap):
                # a,b: [npart, nu2, 16]; out[i,k] = lse_j a[i,j] + b[j,k]
                av = a_ap.rearrange("p u (i j) -> p u i j", i=4)
                bv = b_ap.rearrange("p u (j k) -> p u j k", j=4) \
                    .transpose([0, 1, 3, 2])  # [p, u, k, j]
                xt = tpool.tile([npart, nu2, 4, 4, 4], f32, tag="X", name="X")
                for i in range(4):
                    nc.vector.tensor_tensor(
                        out=xt[:, :, i, :, :],
                        in0=av[:, :, i, :].unsqueeze(2)
                            .broadcast_to([npart, nu2, 4, 4]),
                        in1=bv, op=OP.add)
                mx = tpool.tile([npart, nu2, 4, 4], f32, tag="mx", name="mx")
                nc.vector.tensor_reduce(
                    out=mx[:].rearrange("p u i k -> p (u i k)"),
                    in_=xt[:].rearrange("p u i k j -> p (u i k) j"),
                    axis=AX.X, op=OP.max)
                xs = tpool.tile([npart, nu2, 4, 4, 4], f32, tag="XS", name="XS")
                for i in range(4):
                    nc.vector.tensor_tensor(
                        out=xs[:, :, i, :, :], in0=xt[:, :, i, :, :],
                        in1=mx[:, :, i, :].unsqueeze(3)
                            .broadcast_to([npart, nu2, 4, 4]),
                        op=OP.subtract)
                ex = tpool.tile([npart, nu2, 4, 4, 4], f32, tag="EX", name="EX")
                nc.scalar.activation(
                    ex[:].rearrange("p u i k j -> p (u i k j)"),
                    xs[:].rearrange("p u i k j -> p (u i k j)"), AF.Exp)
                sm = tpool.tile([npart, nu2, 4, 4], f32, tag="sm", name="sm")
                nc.vector.tensor_reduce(
                    out=sm[:].rearrange("p u i k -> p (u i k)"),
                    in_=ex[:].rearrange("p u i k j -> p (u i k) j"),
                    axis=AX.X, op=OP.add)
                ln = tpool.tile([npart, nu2, 4, 4], f32, tag="ln", name="ln")
                nc.scalar.activation(
                    ln[:].rearrange("p u i k -> p (u i k)"),
                    sm[:].rearrange("p u i k -> p (u i k)"), AF.Ln)
                nc.vector.tensor_tensor(
                    out=out_ap,
                    in0=ln[:].rearrange("p u i k -> p u (i k)"),
                    in1=mx[:].rearrange("p u i k -> p u (i k)"), op=OP.add)

            nu = UB
            while nu > 1:
                nxt = tpool.tile([lanes, nu // 2, 16], f32, tag="cur", name="cur")
                combine(cur[:, 0::2, :], cur[:, 1::2, :], lanes, nu // 2, nxt[:])
                cur = nxt[:]
                nu //= 2
            nl = lanes
            while nl > BL:
                half = nl // 2
                dsc = dscr.tile([nl, 16], f32, tag="dsc", name="dsc")
                nc.sync.dma_start(out=dsc, in_=cur[:, 0, :])
                a_t = tpool.tile([half, 1, 16], f32, tag="Ac", name="Ac")
                b_t = tpool.tile([half, 1, 16], f32, tag="Bc", name="Bc")
                nc.sync.dma_start(out=a_t[:, 0, :], in_=dsc[0::2, :])
                nc.sync.dma_start(out=b_t[:, 0, :], in_=dsc[1::2, :])
                nxt = tpool.tile([half, 1, 16], f32, tag="cur", name="cur")
                combine(a_t[:], b_t[:], half, 1, nxt[:])
                cur = nxt[:]
                nl = half

            dsc2 = dscr.tile([lanes, NTAGS], f32, tag="dsc2", name="dsc2")
            nc.sync.dma_start(out=dsc2, in_=em2[:, 0, :])
            em0 = tpool.tile([BL, NTAGS], f32, tag="em0", name="em0")
            nc.sync.dma_start(out=em0, in_=dsc2[0::nq, :])
            a0 = tpool.tile([BL, NTAGS], f32, tag="a0", name="a0")
            nc.vector.tensor_tensor(out=a0, in0=em0, in1=start_sb, op=OP.add)
            y1 = tpool.tile([BL, 16], f32, tag="y1", name="y1")
            nc.vector.tensor_tensor(
                out=y1[:].rearrange("p (i k) -> p i k", i=4),
                in0=cur.rearrange("p u (i k) -> p (u i) k", i=4),
                in1=a0[:].unsqueeze(2).broadcast_to([BL, NTAGS, NTAGS]),
                op=OP.add)
            y2 = tpool.tile([BL, 16], f32, tag="y2", name="y2")
            nc.vector.tensor_tensor(out=y2, in0=y1, in1=end_sb, op=OP.add)
            mxf = tpool.tile([BL, 1], f32, tag="mxf", name="mxf")
            nc.vector.tensor_reduce(out=mxf, in_=y2, axis=AX.X, op=OP.max)
            yd = tpool.tile([BL, 16], f32, tag="yd", name="yd")
            nc.vector.tensor_scalar(out=yd, in0=y2, scalar1=mxf[:], scalar2=None,
                                    op0=OP.subtract)
            ye = tpool.tile([BL, 16], f32, tag="ye", name="ye")
            sme = tpool.tile([BL, 1], f32, tag="sme", name="sme")
            nc.scalar.activation(ye, yd, AF.Exp, accum_out=sme[:])
            lns = tpool.tile([BL, 1], f32, tag="lns", name="lns")
            nc.scalar.activation(lns, sme, AF.Ln)
            lz = tpool.tile([BL, 1], f32, tag="lz", name="lz")
            nc.vector.tensor_tensor(out=lz, in0=lns, in1=mxf, op=OP.add)
            nc.sync.dma_start(out=d_logz, in_=lz)

    nc.compile()
    return nc


# --------------------------------------------------------------------------
# Host preparation
# --------------------------------------------------------------------------

def prep_core_inputs(core, sentence, tags, mask_f, length, embed_full,
                     w_ih, w_hh, w_tag, start_trans, end_trans, trans,
                     s_len=S, sigma_trick=True):
    nq = s_len // UB
    lanes = BL * nq
    ntile = (s_len * BL) // 128
    bsl = slice(core * BL, (core + 1) * BL)
    sent = np.asarray(sentence)[bsl, :s_len]
    tg = np.asarray(tags)[bsl, :s_len]
    mf = np.asarray(mask_f)[bsl, :s_len].astype(np.float32)

    # token gather index: tile T covers t in [16T,16T+16); p = (t%16)*8 + b
    tt = 16 * np.arange(ntile)[None, :] + (np.arange(128) // BL)[:, None]
    bb = (np.arange(128) % BL)[:, None] + np.zeros((1, ntile), np.int64)
    idx = sent[bb, tt].astype(np.int32)

    # gate order (i, f, o, g); reference splits gates as (i, f, g, o)
    perm = np.concatenate([np.arange(0, H2),           # i
                           np.arange(H2, 2 * H2),      # f
                           np.arange(3 * H2, 4 * H2),  # o
                           np.arange(2 * H2, 3 * H2)]) # g
    gscale = np.ones((4 * H2, 1), np.float32)
    if sigma_trick:
        gscale[3 * H2:] = 2.0

    def pack_w(w):  # w [4H2, K] -> [K, 4H2] reordered (+g-scaled)
        wr = (np.asarray(w, np.float32)[perm, :] * gscale)
        return np.ascontiguousarray(wr.T)

    # device h tiles hold h~ = h/2, so every h-consuming weight gets 2x
    whhT = np.stack([np.stack([pack_w(w_hh[l, d]) for d in range(2)])
                     for l in range(NLAYERS)]) * 2.0
    wih0T = np.stack([
        np.stack([pack_w(w_ih[0, d])[kc * 128:(kc + 1) * 128] for kc in range(2)])
        for d in range(2)])
    wih1T = np.stack([
        np.stack([pack_w(w_ih[1, d])[kc * 128:(kc + 1) * 128] for kc in range(2)])
        for d in range(2)]) * 2.0
    wtagT = np.ascontiguousarray(np.asarray(w_tag, np.float32).T) * 2.0
    wtagT = np.stack([wtagT[:128], wtagT[128:]])

    maskf = np.ascontiguousarray(mf.T).reshape(-1)

    tarr = np.arange(s_len)
    qv, uv = tarr // UB, tarr % UB
    gsel = np.zeros((lanes, UB, NTAGS), np.float32)
    msel = np.zeros((lanes, UB), np.float32)
    madd = np.zeros((lanes, UB, 16), np.float32)
    offd = (1.0 - np.eye(NTAGS, dtype=np.float32)).reshape(16)
    for b in range(BL):
        for t in range(s_len):
            lane, u = b * nq + qv[t], uv[t]
            coef = 1.0 if t == 0 else float(mf[b, t])
            gsel[lane, u, int(tg[b, t])] = coef
            valid = (t >= 1) and mf[b, t] > 0
            msel[lane, u] = 1.0 if valid else 0.0
            if not valid:
                madd[lane, u] = NEG * offd

    trans16 = np.ascontiguousarray(np.asarray(trans, np.float32).reshape(16))
    startrep = np.broadcast_to(
        np.asarray(start_trans, np.float32), (BL, NTAGS)).copy()
    endrep = np.broadcast_to(np.asarray(end_trans, np.float32)[None, None, :],
                             (BL, NTAGS, NTAGS)).reshape(BL, 16).copy()
    sel2 = np.zeros((128, BL), np.float32)
    for p in range(lanes):
        sel2[p, p // nq] = 1.0

    import ml_dtypes
    bf = ml_dtypes.bfloat16
    return {
        "embed": embed_full,
        "idx": np.ascontiguousarray(idx),
        "whhT": np.ascontiguousarray(whhT).astype(bf),
        "wih0T": np.ascontiguousarray(wih0T).astype(bf),
        "wih1T": np.ascontiguousarray(wih1T).astype(bf),
        "wtagT": np.ascontiguousarray(wtagT).astype(bf),
        "maskf": maskf,
        "gsel": gsel,
        "msel": msel,
        "madd": madd,
        "trans16": trans16,
        "startrep": startrep,
        "endrep": endrep,
        "sel2": sel2,
    }


def host_trans_score(tags, mask_f, length, start_trans, end_trans, trans):
    tags = np.asarray(tags)
    Bn = tags.shape[0]
    ar = np.arange(Bn)
    sc = np.asarray(start_trans)[tags[:, 0]].astype(np.float64)
    tr = np.asarray(trans)[tags[:, :-1], tags[:, 1:]]
    sc = sc + np.sum(tr * np.asarray(mask_f)[:, 1:], axis=1)
    last = tags[ar, np.asarray(length) - 1]
    sc = sc + np.asarray(end_trans)[last]
    return sc


# --------------------------------------------------------------------------
# Public entry
# --------------------------------------------------------------------------

def kernel(**inputs):
    return _run(inputs, trace=False)[0]


def _run(inputs, trace=False):
    loss, res = _run_impl(trace=trace, **inputs)
    return loss, res


def _run_impl(sentence, tags, mask, length, embed, w_ih, w_hh, b_ih, b_hh,
              w_tag, b_tag, start_trans, end_trans, trans, trace=False):
    from concourse import bass_utils

    sentence = np.asarray(sentence).astype(np.int64)
    tags = np.asarray(tags).astype(np.int64)
    mask_f = np.asarray(mask).astype(np.float32)
    length = np.asarray(length).astype(np.int64)
    embed = np.ascontiguousarray(np.asarray(embed, np.float32))
    w_ih = np.asarray(w_ih, np.float32)
    w_hh = np.asarray(w_hh, np.float32)
    w_tag = np.asarray(w_tag, np.float32)
    start_trans = np.asarray(start_trans, np.float32)
    end_trans = np.asarray(end_trans, np.float32)
    trans = np.asarray(trans, np.float32)

    assert np.all(np.asarray(b_ih) == 0) and np.all(np.asarray(b_hh) == 0) \
        and np.all(np.asarray(b_tag) == 0), "kernel assumes zero biases"

    key = ("prog", S, True)
    if key not in _BUILD_CACHE:
        _BUILD_CACHE[key] = build_program(S, sigma_trick=True)
    nc = _BUILD_CACHE[key]

    in_maps = [prep_core_inputs(core, sentence, tags, mask_f, length, embed,
                                w_ih, w_hh, w_tag, start_trans, end_trans, trans)
               for core in range(NCORES)]

    res = bass_utils.run_bass_kernel_spmd(nc, in_maps, core_ids=list(range(NCORES)),
                                          trace=trace)

    logz = np.concatenate([r["out_logz"] for r in res.results]).astype(np.float64)
    emit = np.concatenate([r["out_emit"] for r in res.results]).astype(np.float64)
    tsc = host_trans_score(tags, mask_f, length, start_trans, end_trans, trans)
    llh = (tsc + emit) - logz
    return np.float32(-np.mean(llh)), res



# revision 8
# speedup vs baseline: 1.2811x; 1.0001x over previous
# BiLSTM-CRF negative log-likelihood on 8 Trainium2 NeuronCores.
# Self-contained: host prep + Bass/Tile device program + unshard.
#
# Sharding: data-parallel over batch. 64 sequences -> 8 cores x 8 seqs.
# Per core: embedding gather -> 2-layer BiLSTM -> tag projection ->
# CRF partition (log-semiring tree reduction) + gold emission dot.
# Host: gold transition score (pure index math on host-visible inputs),
# final llh assembly and mean.

import numpy as np

VOCAB = 50000
EMB = 256
HID = 256
H2 = 128
NLAYERS = 2
NTAGS = 4
B = 64
S = 512
NCORES = 8
BL = B // NCORES          # sequences per core
TB = 16                   # timesteps per x-projection PSUM block
UB = 32                   # CRF tree: timesteps per lane (q = t // UB)
NEG = -1.0e9

_BUILD_CACHE = {}


# --------------------------------------------------------------------------
# Device program
# --------------------------------------------------------------------------

def build_program(s_len=S, sigma_trick=True, n_devices=NCORES):
    """Builds the per-core Bass program (identical on all cores)."""
    import concourse.bacc as bacc
    import concourse.bass as bass
    import concourse.tile as tile
    from concourse import mybir
    from concourse.masks import make_identity
    from contextlib import ExitStack

    f32 = mybir.dt.float32
    bf16 = mybir.dt.bfloat16
    i32 = mybir.dt.int32
    AF = mybir.ActivationFunctionType
    OP = mybir.AluOpType
    AX = mybir.AxisListType

    nq = s_len // UB              # CRF q index = t // UB
    lanes = BL * nq               # CRF lane = b*nq + q  (b-major)
    ntile = (s_len * BL) // 128   # gather tiles of 128 tokens
    nblk = s_len // TB            # recurrence blocks

    nc = bacc.Bacc("TRN2", target_bir_lowering=False, debug=False,
                   enable_asserts=False, num_devices=n_devices)

    # ---- DRAM I/O -------------------------------------------------------
    d_embed = nc.dram_tensor("embed", [VOCAB + 1, EMB], f32, kind="ExternalInput").ap()
    d_idx = nc.dram_tensor("idx", [128, ntile], i32, kind="ExternalInput").ap()
    d_whh = nc.dram_tensor("whhT", [NLAYERS, 2, H2, 4 * H2], bf16, kind="ExternalInput").ap()
    d_wih0 = nc.dram_tensor("wih0T", [2, 2, 128, 4 * H2], bf16, kind="ExternalInput").ap()
    d_wih1 = nc.dram_tensor("wih1T", [2, 2, 128, 4 * H2], bf16, kind="ExternalInput").ap()
    d_wtag = nc.dram_tensor("wtagT", [2, 128, NTAGS], bf16, kind="ExternalInput").ap()
    d_mask = nc.dram_tensor("maskf", [s_len * BL], f32, kind="ExternalInput").ap()
    d_gsel = nc.dram_tensor("gsel", [lanes, UB, NTAGS], f32, kind="ExternalInput").ap()
    d_msel = nc.dram_tensor("msel", [lanes, UB], f32, kind="ExternalInput").ap()
    d_madd = nc.dram_tensor("madd", [lanes, UB, 16], f32, kind="ExternalInput").ap()
    d_trans = nc.dram_tensor("trans16", [16], f32, kind="ExternalInput").ap()
    d_start = nc.dram_tensor("startrep", [BL, NTAGS], f32, kind="ExternalInput").ap()
    d_end = nc.dram_tensor("endrep", [BL, 16], f32, kind="ExternalInput").ap()
    d_sel = nc.dram_tensor("sel2", [128, BL], f32, kind="ExternalInput").ap()

    d_logz = nc.dram_tensor("out_logz", [BL], f32, kind="ExternalOutput").ap()
    d_emit = nc.dram_tensor("out_emit", [BL], f32, kind="ExternalOutput").ap()

    with tile.TileContext(nc) as tc:
        with ExitStack() as ctx:
            consts = ctx.enter_context(tc.tile_pool(name="consts", bufs=1))
            big = ctx.enter_context(tc.tile_pool(name="big", bufs=1))
            work = ctx.enter_context(tc.tile_pool(name="work", bufs=6))
            gpool = ctx.enter_context(tc.tile_pool(name="gath", bufs=3))
            dscr = ctx.enter_context(
                tc.tile_pool(name="dscr", bufs=2, space=bass.MemorySpace.DRAM))
            _b = bass

            # ---- constants into SBUF ------------------------------------
            whh_sb = consts.tile([128, NLAYERS, 2, 4 * H2], bf16, tag="whh", name="whh")
            nc.sync.dma_start(out=whh_sb, in_=d_whh.rearrange("l d k m -> k l d m"))
            wih0_sb = consts.tile([128, 2, 2, 4 * H2], bf16, tag="wih0", name="wih0")
            nc.sync.dma_start(out=wih0_sb, in_=d_wih0.rearrange("d c k m -> k d c m"))
            wih1_sb = consts.tile([128, 2, 2, 4 * H2], bf16, tag="wih1", name="wih1")
            nc.sync.dma_start(out=wih1_sb, in_=d_wih1.rearrange("d c k m -> k d c m"))
            wtag_sb = consts.tile([128, 2, NTAGS], bf16, tag="wtag", name="wtag")
            nc.sync.dma_start(out=wtag_sb, in_=d_wtag.rearrange("c k m -> k c m"))
            idx_sb = consts.tile([128, ntile], i32, tag="idx", name="idx")
            nc.sync.dma_start(out=idx_sb, in_=d_idx)
            sel_sb = consts.tile([128, BL], f32, tag="sel", name="sel")
            nc.sync.dma_start(out=sel_sb, in_=d_sel)
            gsel_sb = consts.tile([lanes, UB, NTAGS], f32, tag="gsel", name="gsel")
            nc.sync.dma_start(out=gsel_sb, in_=d_gsel)
            msel_sb = consts.tile([lanes, UB], f32, tag="msel", name="msel")
            nc.sync.dma_start(out=msel_sb, in_=d_msel)
            madd_sb = consts.tile([lanes, UB, 16], f32, tag="madd", name="madd")
            nc.sync.dma_start(out=madd_sb, in_=d_madd)
            trans_sb = consts.tile([128, 16], f32, tag="trans", name="trans")
            nc.sync.dma_start(
                out=trans_sb,
                in_=_b.AP(tensor=d_trans.tensor, offset=0, ap=[[0, 128], [1, 16]]))
            start_sb = consts.tile([BL, NTAGS], f32, tag="start", name="start")
            nc.sync.dma_start(out=start_sb, in_=d_start)
            end_sb = consts.tile([BL, 16], f32, tag="end", name="end")
            nc.sync.dma_start(out=end_sb, in_=d_end)
            mask_sb = big.tile([128, s_len, BL], f32, tag="mask", name="mask")
            nc.sync.dma_start(
                out=mask_sb,
                in_=_b.AP(tensor=d_mask.tensor, offset=0,
                          ap=[[0, 128], [1, s_len * BL]]))
            ident = consts.tile([128, 128], f32, tag="ident", name="ident")
            make_identity(nc, ident)

            # ---- big state tiles ---------------------------------------
            x0 = big.tile([128, 2, s_len, BL], bf16, tag="x0", name="x0")
            h_layers = [big.tile([128, 2, s_len, BL], bf16, tag=f"H{l}", name=f"H{l}")
                        for l in range(NLAYERS)]
            em2 = big.tile([lanes, UB, NTAGS], f32, tag="em2", name="em2")

            # ---- phase A: embedding gather + transpose ------------------
            with tc.tile_pool(name="psA", bufs=4, space=bass.MemorySpace.PSUM) as psA:
                for T in range(ntile):
                    g_t = gpool.tile([128, EMB], f32, tag="gt", name="gt")
                    nc.gpsimd.indirect_dma_start(
                        out=g_t, out_offset=None,
                        in_=d_embed,
                        in_offset=bass.IndirectOffsetOnAxis(
                            ap=idx_sb[:, T:T + 1], axis=0),
                    )
                    for c in range(2):
                        tp = psA.tile([128, 128], f32, tag="tp", name="tp")
                        nc.tensor.transpose(tp, g_t[:, c * 128:(c + 1) * 128], ident)
                        nc.vector.tensor_copy(
                            x0[:, c, 16 * T:16 * (T + 1), :],
                            tp[:].rearrange("p (t b) -> p t b", b=BL))

            # ---- phases B-D: xproj + recurrence per layer ---------------
            # px tiles are 1 PSUM bank each ([128,4,TB,BL] f32, TB=16);
            # 3-deep rotation per direction so block n+2's x-projection can
            # stream into the PE interleaved with block n's recurrence
            # (fills the PE stall while the cell's ACT/DVE tail runs).
            #
            # Cell math tracks ct = c/2 so the update is a plain sum of four
            # products (no extra affine op for tanh(g)):
            #   ct2 = sf*ct1 + si*sg - si/2
            # with sg = sigmoid(2*gtilde) (g-weights prescaled by 2) laid out
            # as (si,si,sf,sf) (.) (sg, -1/2, ct1, 0) over one sg ring tile:
            # slots 0..3 = sigmoid(gates i,f,o,g), 4 = ct1, 5 = -1/2, 6 = 0.
            # tanh(c) = 2*sigmoid(4*ct)-1 and h~ = h/2 = (sc-1/2)*so in one
            # scalar_tensor_tensor; the missing 2x lives in the host-side
            # prescale of whh/wih1/wtag.
            RING = 6
            cellp = ctx.enter_context(tc.tile_pool(name="cell", bufs=1))
            ring = [[cellp.tile([128, 7, BL], f32, tag=f"ring{d}{r}",
                                name=f"ring{d}{r}") for r in range(RING)]
                    for d in range(2)]
            for d in range(2):
                for r in range(RING):
                    nc.vector.memset(ring[d][r][:, 5, :], -0.5)
                    nc.vector.memset(ring[d][r][:, 6, :], 0.0)
            with tc.tile_pool(name="psR", bufs=3, space=bass.MemorySpace.PSUM) as psR:
                for l in range(NLAYERS):
                    xin = x0 if l == 0 else h_layers[l - 1]
                    hout = h_layers[l]
                    wih = wih0_sb if l == 0 else wih1_sb
                    for d in range(2):
                        for r in range(RING):
                            nc.vector.memset(ring[d][r][:, 4, :], 0.0)

                    def blk_of(d, n):
                        return n if d == 0 else nblk - 1 - n

                    def emit_xproj(d, n, pt, g, kc):
                        blk = blk_of(d, n)
                        nc.tensor.matmul(
                            pt[:, g, :, :],
                            wih[:, d, kc, g * 128:(g + 1) * 128],
                            xin[:, kc, blk * TB:(blk + 1) * TB, :],
                            start=(kc == 0 and g == 0), stop=False,
                            skip_group_check=True)

                    pxq = {0: [], 1: []}
                    for n in range(min(2, nblk)):
                        for d in range(2):
                            pt = psR.tile([128, 4, TB, BL], f32, tag=f"px{d}",
                                          name=f"px{d}")
                            for g in range(4):
                                for kc in range(2):
                                    emit_xproj(d, n, pt, g, kc)
                            pxq[d].append(pt)

                    for blk_i in range(nblk):
                        px = [pxq[0].pop(0), pxq[1].pop(0)]
                        xq = []
                        if blk_i + 2 < nblk:
                            for d in range(2):
                                pt = psR.tile([128, 4, TB, BL], f32, tag=f"px{d}",
                                              name=f"px{d}")
                                pxq[d].append(pt)
                                for g in range(4):
                                    for kc in range(2):
                                        xq.append((d, blk_i + 2, pt, g, kc))
                        for j in range(TB):
                            sgs = [None, None]
                            nxts = [None, None]
                            for d in range(2):
                                blk = blk_of(d, blk_i)
                                t = blk * TB + (j if d == 0 else TB - 1 - j)
                                k = blk_i * TB + j
                                if k > 0:
                                    tprev = t - 1 if d == 0 else t + 1
                                    h_prev = hout[:, d, tprev, :]
                                    for g in range(4):
                                        nc.tensor.matmul(
                                            px[d][:, g, t % TB, :],
                                            whh_sb[:, l, d,
                                                   g * 128:(g + 1) * 128],
                                            h_prev,
                                            start=False,
                                            stop=(j == TB - 1 and g == 3),
                                            skip_group_check=True,
                                        )
                                # one x-projection matmul per dir-step fills
                                # the PE dependency stall with real work
                                if xq:
                                    xd, xn, xpt, xg, xkc = xq.pop(0)
                                    emit_xproj(xd, xn, xpt, xg, xkc)
                                pslice = px[d][:, :, t % TB, :]
                                sg = ring[d][k % RING]
                                nxt = ring[d][(k + 1) % RING]
                                sgs[d] = sg
                                nxts[d] = nxt
                                nc.scalar.activation(sg[:, 0:4, :], pslice,
                                                     AF.Sigmoid)
                                # ct2 = si*sg + si*(-1/2) + sf*ct1 + sf*0
                                prod = work.tile([128, BL, 4], f32,
                                                 tag=f"prod{d}", name=f"prod{d}")
                                in0 = sg[:, 0:2, :].unsqueeze(2) \
                                    .broadcast_to([128, 2, 2, BL])
                                in1 = sg[:, 3:7, :].rearrange(
                                    "p (m0 m1) b -> p m1 m0 b", m0=2)
                                outv = prod[:].rearrange(
                                    "p b (m1 m0) -> p m1 m0 b", m1=2)
                                nc.vector.tensor_tensor(
                                    out=outv, in0=in0, in1=in1, op=OP.mult)
                                if d == 1:
                                    c2tmp = work.tile([128, BL], f32,
                                                      tag=f"c2{d}", name=f"c2{d}")
                                    nc.vector.tensor_reduce(
                                        out=c2tmp, in_=prod[:], axis=AX.X,
                                        op=OP.add)
                                    nc.vector.tensor_tensor(
                                        out=nxt[:, 4, :], in0=c2tmp,
                                        in1=mask_sb[:, t, :], op=OP.mult)
                                else:
                                    nc.vector.tensor_reduce(
                                        out=nxt[:, 4, :], in_=prod[:],
                                        axis=AX.X, op=OP.add)
                            for d in range(2):
                                blk = blk_of(d, blk_i)
                                t = blk * TB + (j if d == 0 else TB - 1 - j)
                                sg = sgs[d]
                                # sc = sigmoid(4*ct2) = (tanh(c2)+1)/2
                                sc = work.tile([128, BL], f32, tag=f"sc{d}",
                                               name=f"sc{d}")
                                nc.scalar.activation(sc, nxts[d][:, 4, :],
                                                     AF.Sigmoid, scale=4.0)
                                # h~ = h/2 = (sc - 1/2) * so
                                nc.vector.scalar_tensor_tensor(
                                    out=hout[:, d, t, :],
                                    in0=sc, scalar=-0.5, in1=sg[:, 2, :],
                                    op0=OP.add, op1=OP.mult)

            # ---- phase E/F: tag projection + emission dot ---------------
            tpool = ctx.enter_context(tc.tile_pool(name="tree", bufs=2))
            nchunk = (s_len * BL) // 128
            emT = big.tile([128, nchunk, NTAGS], f32, tag="emT", name="emT")
            with tc.tile_pool(name="psE", bufs=4, space=bass.MemorySpace.PSUM) as psE:
                h1 = h_layers[NLAYERS - 1]
                for ch in range(nchunk):
                    pe = psE.tile([128, NTAGS], f32, tag="pe", name="pe")
                    for kc in range(2):
                        lhs = h1[:, kc, 16 * ch:16 * (ch + 1), :].rearrange(
                            "p t b -> p (t b)")
                        nc.tensor.matmul(pe, lhs, wtag_sb[:, kc, :],
                                         start=(kc == 0), stop=(kc == 1))
                    nc.vector.tensor_copy(emT[:, ch, :], pe)
                # permute token rows (t*8+b) -> CRF lanes (b*nq+q, u) via DRAM
                demT = dscr.tile([128, nchunk, NTAGS], f32, tag="demT", name="demT")
                nc.sync.dma_start(out=demT, in_=emT)
                # dram addr(p=u0*BL+b, c=2q+u1, j) = p*nchunk*NTAGS + c*NTAGS + j
                dt_ap = demT[:]
                nc.sync.dma_start(
                    out=em2,
                    in_=_b.AP(tensor=dt_ap.tensor, offset=dt_ap.offset,
                              ap=[[nchunk * NTAGS, BL],         # b
                                  [2 * NTAGS, nq],              # q
                                  [NTAGS, 2],                   # u1 = u//16
                                  [BL * nchunk * NTAGS, 16],    # u0 = u%16
                                  [1, NTAGS]]))                 # j

                prod = big.tile([lanes, UB, NTAGS], f32, tag="prod", name="prod")
                nc.vector.tensor_tensor(out=prod, in0=em2, in1=gsel_sb, op=OP.mult)
                rsum = work.tile([lanes, 1], f32, tag="rsum", name="rsum")
                nc.vector.tensor_reduce(out=rsum, in_=prod, axis=AX.XY, op=OP.add)
                pemit = psE.tile([BL, 1], f32, tag="pemit", name="pemit")
                nc.tensor.matmul(pemit, sel_sb[:lanes, :], rsum,
                                 start=True, stop=True)
                emit_sb = work.tile([BL, 1], f32, tag="emit", name="emit")
                nc.vector.tensor_copy(emit_sb, pemit)
                nc.sync.dma_start(out=d_emit, in_=emit_sb)

            # ---- phase G: CRF partition via log-semiring tree -----------
            mten = big.tile([lanes, UB, 16], f32, tag="M", name="M")
            nc.vector.tensor_tensor(
                out=mten[:].rearrange("p u (i j) -> p u i j", i=4),
                in0=trans_sb[:lanes, :].rearrange("p (i j) -> p i j", i=4)
                    .unsqueeze(1).broadcast_to([lanes, UB, NTAGS, NTAGS]),
                in1=em2[:].unsqueeze(2)
                    .broadcast_to([lanes, UB, NTAGS, NTAGS]),
                op=OP.add)
            m2t = big.tile([lanes, UB, 16], f32, tag="M2", name="M2")
            nc.vector.tensor_tensor(
                out=m2t, in0=mten,
                in1=msel_sb[:].unsqueeze(2).broadcast_to([lanes, UB, 16]),
                op=OP.mult)
            cur = big.tile([lanes, UB, 16], f32, tag="M3", name="M3")
            nc.vector.tensor_tensor(out=cur, in0=m2t, in1=madd_sb, op=OP.add)
            cur = cur[:]

            def combine(a_ap, b_ap, npart, nu2, out_ap):
                # a,b: [npart, nu2, 16]; out[i,k] = lse_j a[i,j] + b[j,k]
                av = a_ap.rearrange("p u (i j) -> p u i j", i=4)
                bv = b_ap.rearrange("p u (j k) -> p u j k", j=4) \
                    .transpose([0, 1, 3, 2])  # [p, u, k, j]
                xt = tpool.tile([npart, nu2, 4, 4, 4], f32, tag="X", name="X")
                for i in range(4):
                    nc.vector.tensor_tensor(
                        out=xt[:, :, i, :, :],
                        in0=av[:, :, i, :].unsqueeze(2)
                            .broadcast_to([npart, nu2, 4, 4]),
                        in1=bv, op=OP.add)
                mx = tpool.tile([npart, nu2, 4, 4], f32, tag="mx", name="mx")
                nc.vector.tensor_reduce(
                    out=mx[:].rearrange("p u i k -> p (u i k)"),
                    in_=xt[:].rearrange("p u i k j -> p (u i k) j"),
                    axis=AX.X, op=OP.max)
                xs = tpool.tile([npart, nu2, 4, 4, 4], f32, tag="XS", name="XS")
                for i in range(4):
                    nc.vector.tensor_tensor(
                        out=xs[:, :, i, :, :], in0=xt[:, :, i, :, :],
                        in1=mx[:, :, i, :].unsqueeze(3)
                            .broadcast_to([npart, nu2, 4, 4]),
                        op=OP.subtract)
                ex = tpool.tile([npart, nu2, 4, 4, 4], f32, tag="EX", name="EX")
                nc.scalar.activation(
                    ex[:].rearrange("p u i k j -> p (u i k j)"),
                    xs[:].rearrange("p u i k j -> p (u i k j)"), AF.Exp)
                sm = tpool.tile([npart, nu2, 4, 4], f32, tag="sm", name="sm")
                nc.vector.tensor_reduce(
                    out=sm[:].rearrange("p u i k -> p (u i k)"),
                    in_=ex[:].rearrange("p u i k j -> p (u i k) j"),
                    axis=AX.X, op=OP.add)
                ln = tpool.tile([npart, nu2, 4, 4], f32, tag="ln", name="ln")
                nc.scalar.activation(
                    ln[:].rearrange("p u i k -> p (u i k)"),
                    sm[:].rearrange("p u i k -> p (u i k)"), AF.Ln)
                nc.vector.tensor_tensor(
                    out=out_ap,
                    in0=ln[:].rearrange("p u i k -> p u (i k)"),
                    in1=mx[:].rearrange("p u i k -> p u (i k)"), op=OP.add)

            nu = UB
            while nu > 1:
                nxt = tpool.tile([lanes, nu // 2, 16], f32, tag="cur", name="cur")
                combine(cur[:, 0::2, :], cur[:, 1::2, :], lanes, nu // 2, nxt[:])
                cur = nxt[:]
                nu //= 2
            nl = lanes
            while nl > BL:
                half = nl // 2
                dsc = dscr.tile([nl, 16], f32, tag="dsc", name="dsc")
                nc.sync.dma_start(out=dsc, in_=cur[:, 0, :])
                a_t = tpool.tile([half, 1, 16], f32, tag="Ac", name="Ac")
                b_t = tpool.tile([half, 1, 16], f32, tag="Bc", name="Bc")
                nc.sync.dma_start(out=a_t[:, 0, :], in_=dsc[0::2, :])
                nc.sync.dma_start(out=b_t[:, 0, :], in_=dsc[1::2, :])
                nxt = tpool.tile([half, 1, 16], f32, tag="cur", name="cur")
                combine(a_t[:], b_t[:], half, 1, nxt[:])
                cur = nxt[:]
                nl = half

            dsc2 = dscr.tile([lanes, NTAGS], f32, tag="dsc2", name="dsc2")
            nc.sync.dma_start(out=dsc2, in_=em2[:, 0, :])
            em0 = tpool.tile([BL, NTAGS], f32, tag="em0", name="em0")
            nc.sync.dma_start(out=em0, in_=dsc2[0::nq, :])
            a0 = tpool.tile([BL, NTAGS], f32, tag="a0", name="a0")
            nc.vector.tensor_tensor(out=a0, in0=em0, in1=start_sb, op=OP.add)
            y1 = tpool.tile([BL, 16], f32, tag="y1", name="y1")
            nc.vector.tensor_tensor(
                out=y1[:].rearrange("p (i k) -> p i k", i=4),
                in0=cur.rearrange("p u (i k) -> p (u i) k", i=4),
                in1=a0[:].unsqueeze(2).broadcast_to([BL, NTAGS, NTAGS]),
                op=OP.add)
            y2 = tpool.tile([BL, 16], f32, tag="y2", name="y2")
            nc.vector.tensor_tensor(out=y2, in0=y1, in1=end_sb, op=OP.add)
            mxf = tpool.tile([BL, 1], f32, tag="mxf", name="mxf")
            nc.vector.tensor_reduce(out=mxf, in_=y2, axis=AX.X, op=OP.max)
            yd = tpool.tile([BL, 16], f32, tag="yd", name="yd")
            nc.vector.tensor_scalar(out=yd, in0=y2, scalar1=mxf[:], scalar2=None,
                                    op0=OP.subtract)
            ye = tpool.tile([BL, 16], f32, tag="ye", name="ye")
            sme = tpool.tile([BL, 1], f32, tag="sme", name="sme")
            nc.scalar.activation(ye, yd, AF.Exp, accum_out=sme[:])
            lns = tpool.tile([BL, 1], f32, tag="lns", name="lns")
            nc.scalar.activation(lns, sme, AF.Ln)
            lz = tpool.tile([BL, 1], f32, tag="lz", name="lz")
            nc.vector.tensor_tensor(out=lz, in0=lns, in1=mxf, op=OP.add)
            nc.sync.dma_start(out=d_logz, in_=lz)

    nc.compile()
    return nc


# --------------------------------------------------------------------------
# Host preparation
# --------------------------------------------------------------------------

def prep_core_inputs(core, sentence, tags, mask_f, length, embed_full,
                     w_ih, w_hh, w_tag, start_trans, end_trans, trans,
                     s_len=S, sigma_trick=True):
    nq = s_len // UB
    lanes = BL * nq
    ntile = (s_len * BL) // 128
    bsl = slice(core * BL, (core + 1) * BL)
    sent = np.asarray(sentence)[bsl, :s_len]
    tg = np.asarray(tags)[bsl, :s_len]
    mf = np.asarray(mask_f)[bsl, :s_len].astype(np.float32)

    # token gather index: tile T covers t in [16T,16T+16); p = (t%16)*8 + b
    tt = 16 * np.arange(ntile)[None, :] + (np.arange(128) // BL)[:, None]
    bb = (np.arange(128) % BL)[:, None] + np.zeros((1, ntile), np.int64)
    idx = sent[bb, tt].astype(np.int32)

    # gate order (i, f, o, g); reference splits gates as (i, f, g, o)
    perm = np.concatenate([np.arange(0, H2),           # i
                           np.arange(H2, 2 * H2),      # f
                           np.arange(3 * H2, 4 * H2),  # o
                           np.arange(2 * H2, 3 * H2)]) # g
    gscale = np.ones((4 * H2, 1), np.float32)
    if sigma_trick:
        gscale[3 * H2:] = 2.0

    def pack_w(w):  # w [4H2, K] -> [K, 4H2] reordered (+g-scaled)
        wr = (np.asarray(w, np.float32)[perm, :] * gscale)
        return np.ascontiguousarray(wr.T)

    # device h tiles hold h~ = h/2, so every h-consuming weight gets 2x
    whhT = np.stack([np.stack([pack_w(w_hh[l, d]) for d in range(2)])
                     for l in range(NLAYERS)]) * 2.0
    wih0T = np.stack([
        np.stack([pack_w(w_ih[0, d])[kc * 128:(kc + 1) * 128] for kc in range(2)])
        for d in range(2)])
    wih1T = np.stack([
        np.stack([pack_w(w_ih[1, d])[kc * 128:(kc + 1) * 128] for kc in range(2)])
        for d in range(2)]) * 2.0
    wtagT = np.ascontiguousarray(np.asarray(w_tag, np.float32).T) * 2.0
    wtagT = np.stack([wtagT[:128], wtagT[128:]])

    maskf = np.ascontiguousarray(mf.T).reshape(-1)

    tarr = np.arange(s_len)
    qv, uv = tarr // UB, tarr % UB
    gsel = np.zeros((lanes, UB, NTAGS), np.float32)
    msel = np.zeros((lanes, UB), np.float32)
    madd = np.zeros((lanes, UB, 16), np.float32)
    offd = (1.0 - np.eye(NTAGS, dtype=np.float32)).reshape(16)
    for b in range(BL):
        for t in range(s_len):
            lane, u = b * nq + qv[t], uv[t]
            coef = 1.0 if t == 0 else float(mf[b, t])
            gsel[lane, u, int(tg[b, t])] = coef
            valid = (t >= 1) and mf[b, t] > 0
            msel[lane, u] = 1.0 if valid else 0.0
            if not valid:
                madd[lane, u] = NEG * offd

    trans16 = np.ascontiguousarray(np.asarray(trans, np.float32).reshape(16))
    startrep = np.broadcast_to(
        np.asarray(start_trans, np.float32), (BL, NTAGS)).copy()
    endrep = np.broadcast_to(np.asarray(end_trans, np.float32)[None, None, :],
                             (BL, NTAGS, NTAGS)).reshape(BL, 16).copy()
    sel2 = np.zeros((128, BL), np.float32)
    for p in range(lanes):
        sel2[p, p // nq] = 1.0

    import ml_dtypes
    bf = ml_dtypes.bfloat16
    return {
        "embed": embed_full,
        "idx": np.ascontiguousarray(idx),
        "whhT": np.ascontiguousarray(whhT).astype(bf),
        "wih0T": np.ascontiguousarray(wih0T).astype(bf),
        "wih1T": np.ascontiguousarray(wih1T).astype(bf),
        "wtagT": np.ascontiguousarray(wtagT).astype(bf),
        "maskf": maskf,
        "gsel": gsel,
        "msel": msel,
        "madd": madd,
        "trans16": trans16,
        "startrep": startrep,
        "endrep": endrep,
        "sel2": sel2,
    }


def host_trans_score(tags, mask_f, length, start_trans, end_trans, trans):
    tags = np.asarray(tags)
    Bn = tags.shape[0]
    ar = np.arange(Bn)
    sc = np.asarray(start_trans)[tags[:, 0]].astype(np.float64)
    tr = np.asarray(trans)[tags[:, :-1], tags[:, 1:]]
    sc = sc + np.sum(tr * np.asarray(mask_f)[:, 1:], axis=1)
    last = tags[ar, np.asarray(length) - 1]
    sc = sc + np.asarray(end_trans)[last]
    return sc


# --------------------------------------------------------------------------
# Public entry
# --------------------------------------------------------------------------

def kernel(**inputs):
    return _run(inputs, trace=False)[0]


def _run(inputs, trace=False):
    loss, res = _run_impl(trace=trace, **inputs)
    return loss, res


def _run_impl(sentence, tags, mask, length, embed, w_ih, w_hh, b_ih, b_hh,
              w_tag, b_tag, start_trans, end_trans, trans, trace=False):
    from concourse import bass_utils

    sentence = np.asarray(sentence).astype(np.int64)
    tags = np.asarray(tags).astype(np.int64)
    mask_f = np.asarray(mask).astype(np.float32)
    length = np.asarray(length).astype(np.int64)
    embed = np.ascontiguousarray(np.asarray(embed, np.float32))
    w_ih = np.asarray(w_ih, np.float32)
    w_hh = np.asarray(w_hh, np.float32)
    w_tag = np.asarray(w_tag, np.float32)
    start_trans = np.asarray(start_trans, np.float32)
    end_trans = np.asarray(end_trans, np.float32)
    trans = np.asarray(trans, np.float32)

    assert np.all(np.asarray(b_ih) == 0) and np.all(np.asarray(b_hh) == 0) \
        and np.all(np.asarray(b_tag) == 0), "kernel assumes zero biases"

    key = ("prog", S, True)
    if key not in _BUILD_CACHE:
        _BUILD_CACHE[key] = build_program(S, sigma_trick=True)
    nc = _BUILD_CACHE[key]

    in_maps = [prep_core_inputs(core, sentence, tags, mask_f, length, embed,
                                w_ih, w_hh, w_tag, start_trans, end_trans, trans)
               for core in range(NCORES)]

    res = bass_utils.run_bass_kernel_spmd(nc, in_maps, core_ids=list(range(NCORES)),
                                          trace=trace)

    logz = np.concatenate([r["out_logz"] for r in res.results]).astype(np.float64)
    emit = np.concatenate([r["out_emit"] for r in res.results]).astype(np.float64)
    tsc = host_trans_score(tags, mask_f, length, start_trans, end_trans, trans)
    llh = (tsc + emit) - logz
    return np.float32(-np.mean(llh)), res

